# revision 12
# baseline (speedup 1.0000x reference)
"""Trainium2 Bass kernel for nn_AugmentShallow (gnn_message_passing).

Reference computation (per batch b):
    g  = x[b, knn_idx[b]]                       # [N, K, 3] gather
    h  = g @ W1.T + b1                          # [N, K, 128]
    h  = relu(h @ Wc0.T + bc0)                  # [N, K, 128]
    h  = relu(h @ Wc1.T + bc1)                  # [N, K, 128]
    m  = mean_k h                               # [N, 128]
    out = m @ W2.T + b2                         # [N, 256]

Strength reduction: every per-(n,k) value depends only on the gathered
point index j = knn_idx[b,n,k], so the MLP runs once per unique point
(N rows instead of N*K):
    p[j] = relu(Weff @ x[j] + beff)   with Weff = Wc0 @ W1 (host-fused)
    q[j] = relu(Wc1 @ p[j] + bc1)
    m[n] = sum_k q[knn[n,k]]          (1/K folded into W2)
    out  = m @ (W2/K).T + b2

The q-row gather runs as SWDGE dma_gather (non-transpose, DRAM source)
split across all 4 SWDGE queues — each queue runs on its own Q7 core
pair, so descriptor generation (the bottleneck) is 4x parallel. The
transpose-mode gather cannot be multi-queued (concurrent xbar streams
corrupt each other), which is why q is staged through DRAM row-major
and gathered token-major. The K-sum is PE identity-matmul PSUM
accumulation; m tiles are DVE-transposed to channel-major for trans2.

Sharding: data-parallel over B — core i owns batch i (knn_idx[b] only
references batch b, so no cross-core traffic).
"""

import sys

if "/opt/trn_rl_repo" not in sys.path:
    sys.path.insert(0, "/opt/trn_rl_repo")

import numpy as np

B, N, K = 8, 8192, 12
C_IN, C_HID, C_OUT = 3, 128, 256

CHUNK = 512                     # output tokens per gather chunk
N_CHUNKS = N // CHUNK           # 16
IDX_PER_CHUNK = CHUNK * K       # 6144
IDX_SLOTS = IDX_PER_CHUNK // 16  # 384 (16-partition wrap)
G_SLOTS = IDX_PER_CHUNK // 128  # 48
P_TOK = 512                     # tokens per p-stage matmul
N_QUEUES = 4                    # SWDGE queues (parallel Q7 desc-gen pairs)

_CACHE = {}


def _build_program():
    import concourse.bacc as bacc
    import concourse.mybir as mybir
    import concourse.tile as tile

    dt = mybir.dt
    nc = bacc.Bacc("TRN2", target_bir_lowering=False, debug=False, num_devices=8,
                   num_swdge_queues=N_QUEUES)

    xT_d = nc.dram_tensor("xT", [C_IN, N], dt.float32, kind="ExternalInput")
    idx_d = nc.dram_tensor("idx", [128, N_CHUNKS * IDX_SLOTS], dt.int16,
                           kind="ExternalInput")
    weffT_d = nc.dram_tensor("weffT", [C_IN, C_HID], dt.float32,
                             kind="ExternalInput")
    beff_d = nc.dram_tensor("beff", [C_HID, 1], dt.float32, kind="ExternalInput")
    wc1T_d = nc.dram_tensor("wc1T", [C_HID, C_HID], dt.float16,
                            kind="ExternalInput")
    bc1_d = nc.dram_tensor("bc1r", [1, C_HID], dt.float16, kind="ExternalInput")
    w2T_d = nc.dram_tensor("w2T", [C_HID, C_OUT], dt.float16,
                           kind="ExternalInput")
    b2_d = nc.dram_tensor("b2r", [1, C_OUT], dt.float16, kind="ExternalInput")
    ident_d = nc.dram_tensor("ident", [128, 128], dt.float16,
                             kind="ExternalInput")
    ones_d = nc.dram_tensor("ones", [1, 128], dt.float16, kind="ExternalInput")
    out_d = nc.dram_tensor("out", [N, C_OUT], dt.float32, kind="ExternalOutput")

    with tile.TileContext(nc) as tc:
        with (
            tc.tile_pool(name="const", bufs=1) as cpool,
            tc.tile_pool(name="qstage", bufs=3) as qpool,
            tc.tile_pool(name="gpool", bufs=6) as gpool,
            tc.tile_pool(name="mpool", bufs=3) as mpool,
            tc.tile_pool(name="mtpool", bufs=3) as mtpool,
            tc.tile_pool(name="opool", bufs=2) as opool,
            tc.tile_pool(name="qdram", bufs=1, space="DRAM") as dpool,
            tc.tile_pool(name="pp", bufs=2, space="PSUM") as pp,
            tc.tile_pool(name="pq", bufs=2, space="PSUM") as pq,
            tc.tile_pool(name="pt", bufs=2, space="PSUM") as pt,
            tc.tile_pool(name="po", bufs=2, space="PSUM") as po,
        ):
            # ---- persistent SBUF tensors -------------------------------
            xT = cpool.tile([C_IN, N], dt.float32)
            idx = cpool.tile([128, N_CHUNKS * IDX_SLOTS], dt.int16)
            weffT = cpool.tile([C_IN, C_HID], dt.float32)
            beff = cpool.tile([C_HID, 1], dt.float32)
            wc1T = cpool.tile([C_HID, C_HID], dt.float16)
            bc1 = cpool.tile([1, C_HID], dt.float16)
            w2T = cpool.tile([C_HID, C_OUT], dt.float16)
            b2 = cpool.tile([1, C_OUT], dt.float16)
            ident = cpool.tile([128, 128], dt.float16)
            ones = cpool.tile([1, 128], dt.float16)
            p_f16 = cpool.tile([128, N], dt.float16)   # [ch, tok]
            q_dram = dpool.tile([N, C_HID], dt.float16)  # token-major rows

            nc.sync.dma_start(xT[:], xT_d.ap()[:])
            nc.sync.dma_start(idx[:], idx_d.ap()[:])
            nc.sync.dma_start(weffT[:], weffT_d.ap()[:])
            nc.sync.dma_start(beff[:], beff_d.ap()[:])
            nc.sync.dma_start(wc1T[:], wc1T_d.ap()[:])
            nc.sync.dma_start(bc1[:], bc1_d.ap()[:])
            nc.sync.dma_start(w2T[:], w2T_d.ap()[:])
            nc.sync.dma_start(b2[:], b2_d.ap()[:])
            nc.sync.dma_start(ident[:], ident_d.ap()[:])
            nc.sync.dma_start(ones[:], ones_d.ap()[:])

            # ---- p = relu(Weff @ x + beff), channel-major [128, N] -----
            for c in range(N // P_TOK):
                ppt = pp.tile([128, P_TOK], dt.float32, tag="ps512")
                nc.tensor.matmul(
                    ppt[:], weffT[:], xT[:, c * P_TOK:(c + 1) * P_TOK],
                    start=True, stop=True,
                )
                nc.scalar.activation(
                    p_f16[:, c * P_TOK:(c + 1) * P_TOK], ppt[:],
                    mybir.ActivationFunctionType.Relu, bias=beff[:],
                )

            # ---- q = relu(Wc1 @ p + bc1), token-major -> DRAM ----------
            # psum[tok, ch] = p_tile.T @ wc1T (+ ones.T @ bc1)
            for g4 in range(N // P_TOK):
                qsb = qpool.tile([128, P_TOK], dt.float16)
                for s in range(P_TOK // 128):
                    t = g4 * (P_TOK // 128) + s
                    qpt = pq.tile([128, 128], dt.float32)
                    nc.tensor.matmul(
                        qpt[:], p_f16[:, t * 128:(t + 1) * 128], wc1T[:],
                        start=True, stop=False,
                    )
                    nc.tensor.matmul(qpt[:], ones[:], bc1[:],
                                     start=False, stop=True)
                    nc.vector.tensor_relu(qsb[:, s * 128:(s + 1) * 128], qpt[:])
                nc.sync.dma_start(
                    q_dram[g4 * P_TOK:(g4 + 1) * P_TOK, :]
                    .rearrange("(s p) o -> p s o", p=128),
                    qsb[:].rearrange("p (s o) -> p s o", o=C_HID),
                )

            # ---- gather + K-sum + trans2, chunked ----------------------
            for c in range(N_CHUNKS):
                g = gpool.tile([128, G_SLOTS, 128], dt.float16)
                nc.gpsimd.dma_gather(
                    g[:],
                    q_dram[:],
                    idx[:, c * IDX_SLOTS:(c + 1) * IDX_SLOTS],
                    num_idxs=IDX_PER_CHUNK,
                    num_idxs_reg=IDX_PER_CHUNK,
                    elem_size=C_HID,
                    transpose=False,
                    single_packet=False,
                    queue_num=c % N_QUEUES,
                )
                # K-sum: psum[tok128, 4*128ch] += I.T @ G_k   (I.T @ X = X)
                mps = pp.tile([128, CHUNK], dt.float32, tag="ps512")
                for kb in range(K):
                    nc.tensor.matmul(
                        mps[:],
                        ident[:],
                        g[:, kb * (CHUNK // 128):(kb + 1) * (CHUNK // 128), :],
                        start=(kb == 0), stop=(kb == K - 1),
                    )
                m_f16 = mpool.tile([128, CHUNK], dt.float16)  # token-major
                nc.vector.tensor_copy(m_f16[:], mps[:])

                osb = opool.tile([128, CHUNK // 128 * C_OUT], dt.float32)
                for s in range(CHUNK // 128):
                    tps = pt.tile([128, 128], dt.float16)
                    nc.tensor.transpose(
                        tps[:], m_f16[:, s * 128:(s + 1) * 128], ident[:])
                    mt = mtpool.tile([128, 128], dt.float16)  # [ch, tok]
                    nc.vector.tensor_copy(mt[:], tps[:])
                    ops = po.tile([128, C_OUT], dt.float32)
                    nc.tensor.matmul(ops[:], mt[:], w2T[:],
                                     start=True, stop=False)
                    nc.tensor.matmul(ops[:], ones[:], b2[:],
                                     start=False, stop=True)
                    nc.vector.tensor_copy(
                        osb[:, s * C_OUT:(s + 1) * C_OUT], ops[:])
                nc.sync.dma_start(
                    out_d.ap()[c * CHUNK:(c + 1) * CHUNK, :]
                    .rearrange("(s p) o -> p s o", p=128),
                    osb[:].rearrange("p (s o) -> p s o", o=C_OUT),
                )

    nc.compile()
    return nc


def _get_program():
    if "nc" not in _CACHE:
        _CACHE["nc"] = _build_program()
    return _CACHE["nc"]


def _host_prep(x, knn_idx, W1, b1, Wc0, bc0, Wc1, bc1, W2, b2):
    """Fuse weights and build per-core input maps."""
    f64 = np.float64
    weff = (Wc0.astype(f64) @ W1.astype(f64))                    # [128, 3]
    beff = (Wc0.astype(f64) @ b1.astype(f64) + bc0.astype(f64))  # [128]
    w2s = W2.astype(f64) / K                                     # fold 1/K

    weffT = np.ascontiguousarray(weff.T.astype(np.float32))
    beff_c = np.ascontiguousarray(beff.astype(np.float32)[:, None])
    wc1T = np.ascontiguousarray(Wc1.T.astype(np.float16))
    bc1_r = np.ascontiguousarray(bc1.astype(np.float16)[None, :])
    w2T = np.ascontiguousarray(w2s.T.astype(np.float16))
    b2_r = np.ascontiguousarray(b2.astype(np.float16)[None, :])
    ident = np.eye(128, dtype=np.float16)
    ones = np.ones((1, 128), dtype=np.float16)

    in_maps = []
    for bi in range(B):
        xT = np.ascontiguousarray(x[bi].T.astype(np.float32))
        # idx layout: per chunk, k-major flat list wrapped into 16
        # partitions, replicated to all 8 Q7 core groups (128 partitions).
        kb = knn_idx[bi].astype(np.int16)
        cols = []
        for c in range(N_CHUNKS):
            flat = np.ascontiguousarray(
                kb[c * CHUNK:(c + 1) * CHUNK, :].T).reshape(-1)  # k-major
            wrapped = flat.reshape(IDX_SLOTS, 16).T
            cols.append(np.tile(wrapped, (8, 1)))
        idx = np.ascontiguousarray(np.concatenate(cols, axis=1))
        in_maps.append({
            "xT": xT, "idx": idx, "weffT": weffT, "beff": beff_c,
            "wc1T": wc1T, "bc1r": bc1_r, "w2T": w2T, "b2r": b2_r,
            "ident": ident, "ones": ones,
        })
    return in_maps


def kernel(x, knn_idx, W1, b1, Wc0, bc0, Wc1, bc1, W2, b2):
    x = np.asarray(x)
    knn_idx = np.asarray(knn_idx)
    args = [np.asarray(a) for a in (W1, b1, Wc0, bc0, Wc1, bc1, W2, b2)]
    in_maps = _host_prep(x, knn_idx, *args)
    nc = _get_program()
    from concourse import bass_utils
    res = bass_utils.run_bass_kernel_spmd(nc, in_maps, core_ids=list(range(B)))
    return np.stack([res.results[i]["out"] for i in range(B)], axis=0)


# revision 19
# speedup vs baseline: 1.1194x; 1.1194x over previous
"""Trainium2 Bass kernel for nn_AugmentShallow (gnn_message_passing).

Reference computation (per batch b):
    g  = x[b, knn_idx[b]]                       # [N, K, 3] gather
    h  = g @ W1.T + b1                          # [N, K, 128]
    h  = relu(h @ Wc0.T + bc0)                  # [N, K, 128]
    h  = relu(h @ Wc1.T + bc1)                  # [N, K, 128]
    m  = mean_k h                               # [N, 128]
    out = m @ W2.T + b2                         # [N, 256]

Strength reduction: every per-(n,k) value depends only on the gathered
point index j = knn_idx[b,n,k], so the MLP runs once per unique point
(N rows instead of N*K):
    p[j] = relu(Weff @ x[j] + beff)   with Weff = Wc0 @ W1 (host-fused)
    q[j] = relu(Wc1 @ p[j] + bc1)
    m[n] = sum_k q[knn[n,k]]          (1/K folded into W2)
    out  = m @ (W2/K).T + b2

The q-row gather runs as SWDGE dma_gather (non-transpose, DRAM source)
split across all 4 SWDGE queues — each queue runs on its own Q7 core
pair, so descriptor generation (the bottleneck) is 4x parallel. The
transpose-mode gather cannot be multi-queued (concurrent xbar streams
corrupt each other), which is why q is staged through DRAM row-major
and gathered token-major. The K-sum is PE identity-matmul PSUM
accumulation; m tiles are DVE-transposed to channel-major for trans2.

Sharding: data-parallel over B — core i owns batch i (knn_idx[b] only
references batch b, so no cross-core traffic).
"""

import sys

if "/opt/trn_rl_repo" not in sys.path:
    sys.path.insert(0, "/opt/trn_rl_repo")

import numpy as np

B, N, K = 8, 8192, 12
C_IN, C_HID, C_OUT = 3, 128, 256

CHUNK = 512                     # output tokens per gather chunk
N_CHUNKS = N // CHUNK           # 16
IDX_PER_CHUNK = CHUNK * K       # 6144
IDX_SLOTS = IDX_PER_CHUNK // 16  # 384 (16-partition wrap)
G_SLOTS = IDX_PER_CHUNK // 128  # 48
P_TOK = 512                     # tokens per p-stage matmul
N_QUEUES = 4                    # SWDGE queues (parallel Q7 desc-gen pairs)

_CACHE = {}


def _build_program():
    import concourse.bacc as bacc
    import concourse.mybir as mybir
    import concourse.tile as tile

    dt = mybir.dt
    nc = bacc.Bacc("TRN2", target_bir_lowering=False, debug=False, num_devices=8,
                   num_swdge_queues=N_QUEUES)

    xT_d = nc.dram_tensor("xT", [C_IN, N], dt.float16, kind="ExternalInput")
    idx_d = nc.dram_tensor("idx", [128, N_CHUNKS * IDX_SLOTS], dt.int16,
                           kind="ExternalInput")
    weffT_d = nc.dram_tensor("weffT", [C_IN, C_HID], dt.float16,
                             kind="ExternalInput")
    beff_d = nc.dram_tensor("beff", [C_HID, 1], dt.float32, kind="ExternalInput")
    wc1T_d = nc.dram_tensor("wc1T", [C_HID, C_HID], dt.float16,
                            kind="ExternalInput")
    bc1_d = nc.dram_tensor("bc1c", [C_HID, 1], dt.float32, kind="ExternalInput")
    w2T_d = nc.dram_tensor("w2T", [C_HID, C_OUT], dt.float16,
                           kind="ExternalInput")
    b2_d = nc.dram_tensor("b2b", [128, C_OUT], dt.float32, kind="ExternalInput")
    ident_d = nc.dram_tensor("ident", [128, 128], dt.float16,
                             kind="ExternalInput")
    ones_d = nc.dram_tensor("ones", [1, 128], dt.float16, kind="ExternalInput")
    out_d = nc.dram_tensor("out", [N, C_OUT], dt.float32, kind="ExternalOutput")

    with tile.TileContext(nc) as tc:
        with (
            tc.tile_pool(name="const", bufs=1) as cpool,
            tc.tile_pool(name="qstage", bufs=4) as qpool,
            tc.tile_pool(name="gpool", bufs=9) as gpool,
            tc.tile_pool(name="mpool", bufs=3) as mpool,
            tc.tile_pool(name="mtpool", bufs=3) as mtpool,
            tc.tile_pool(name="opool", bufs=2) as opool,
            tc.tile_pool(name="qdram", bufs=1, space="DRAM") as dpool,
            tc.tile_pool(name="pp", bufs=3, space="PSUM") as pp,
            tc.tile_pool(name="pt", bufs=3, space="PSUM") as pt,
            tc.tile_pool(name="po", bufs=2, space="PSUM") as po,
        ):
            # ---- persistent SBUF tensors -------------------------------
            xT = cpool.tile([C_IN, N], dt.float16)
            idx = cpool.tile([128, N_CHUNKS * IDX_SLOTS], dt.int16)
            weffT = cpool.tile([C_IN, C_HID], dt.float16)
            beff = cpool.tile([C_HID, 1], dt.float32)
            wc1T = cpool.tile([C_HID, C_HID], dt.float16)
            bc1 = cpool.tile([C_HID, 1], dt.float32)
            w2T = cpool.tile([C_HID, C_OUT], dt.float16)
            b2 = cpool.tile([128, C_OUT], dt.float32)
            ident = cpool.tile([128, 128], dt.float16)
            ones = cpool.tile([1, 128], dt.float16)
            p_f16 = cpool.tile([128, N], dt.float16)   # [ch, tok]
            q_dram = dpool.tile([N, C_HID], dt.float16)  # token-major rows

            nc.sync.dma_start(xT[:], xT_d.ap()[:])
            nc.sync.dma_start(weffT[:], weffT_d.ap()[:])
            nc.sync.dma_start(beff[:], beff_d.ap()[:])
            nc.sync.dma_start(wc1T[:], wc1T_d.ap()[:])
            nc.sync.dma_start(bc1[:], bc1_d.ap()[:])
            nc.sync.dma_start(w2T[:], w2T_d.ap()[:])
            nc.sync.dma_start(b2[:], b2_d.ap()[:])
            nc.sync.dma_start(ident[:], ident_d.ap()[:])
            nc.sync.dma_start(ones[:], ones_d.ap()[:])
            nc.sync.dma_start(idx[:], idx_d.ap()[:])

            # ---- p = relu(Weff @ x + beff), channel-major [128, N] -----
            for c in range(N // P_TOK):
                ppt = pp.tile([128, P_TOK], dt.float32, tag="ps512")
                nc.tensor.matmul(
                    ppt[:], weffT[:], xT[:, c * P_TOK:(c + 1) * P_TOK],
                    start=True, stop=True,
                )
                nc.scalar.activation(
                    p_f16[:, c * P_TOK:(c + 1) * P_TOK], ppt[:],
                    mybir.ActivationFunctionType.Relu, bias=beff[:],
                )

            # ---- q = relu(Wc1 @ p + bc1) -------------------------------
            # channel-major matmul (constant stationary), ACT bias+relu,
            # then PE-transpose each 128-tile to token-major rows for DRAM.
            for g4 in range(N // P_TOK):
                qps = pp.tile([128, P_TOK], dt.float32, tag="ps512")
                nc.tensor.matmul(
                    qps[:], wc1T[:], p_f16[:, g4 * P_TOK:(g4 + 1) * P_TOK],
                    start=True, stop=True,
                )
                q_cm = qpool.tile([128, P_TOK], dt.float16, tag="qcm")
                nc.scalar.activation(
                    q_cm[:], qps[:],
                    mybir.ActivationFunctionType.Relu, bias=bc1[:],
                )
                qsb = qpool.tile([128, P_TOK], dt.float16, tag="qsb")
                for s in range(P_TOK // 128):
                    tq = pt.tile([128, 128], dt.float16, tag="tps")
                    nc.tensor.transpose(
                        tq[:], q_cm[:, s * 128:(s + 1) * 128], ident[:])
                    nc.vector.tensor_copy(qsb[:, s * 128:(s + 1) * 128], tq[:])
                nc.sync.dma_start(
                    q_dram[g4 * P_TOK:(g4 + 1) * P_TOK, :]
                    .rearrange("(s p) o -> p s o", p=128),
                    qsb[:].rearrange("p (s o) -> p s o", o=C_HID),
                )

            # ---- gather + K-sum + trans2, chunked ----------------------
            for c in range(N_CHUNKS):
                g = gpool.tile([128, G_SLOTS, 128], dt.float16)
                nc.gpsimd.dma_gather(
                    g[:],
                    q_dram[:],
                    idx[:, c * IDX_SLOTS:(c + 1) * IDX_SLOTS],
                    num_idxs=IDX_PER_CHUNK,
                    num_idxs_reg=IDX_PER_CHUNK,
                    elem_size=C_HID,
                    transpose=False,
                    single_packet=False,
                    queue_num=c % N_QUEUES,
                )
                # K-sum: k 0..7 accumulate on PE (psum += I.T @ G_k, and
                # I.T @ X = X); k 8..11 as one DVE strided reduce; the
                # PSUM->SBUF copy doubles as the combine add.
                m_f16 = mpool.tile([128, CHUNK], dt.float16)  # token-major
                part = mpool.tile([128, CHUNK], dt.float32, tag="part")
                nc.vector.reduce_sum(
                    part[:],
                    g[:, 8 * (CHUNK // 128):, :]
                    .rearrange("p (k t) c -> p t c k", k=4),
                    axis=mybir.AxisListType.X,
                )
                mps = pp.tile([128, CHUNK], dt.float32, tag="ps512")
                for kb in range(8):
                    nc.tensor.matmul(
                        mps[:],
                        ident[:],
                        g[:, kb * (CHUNK // 128):(kb + 1) * (CHUNK // 128), :],
                        start=(kb == 0), stop=(kb == 7),
                    )
                nc.vector.tensor_add(m_f16[:], mps[:], part[:])

                osb = opool.tile([128, CHUNK // 128 * C_OUT], dt.float32)
                for s in range(CHUNK // 128):
                    tps = pt.tile([128, 128], dt.float16, tag="tps")
                    nc.tensor.transpose(
                        tps[:], m_f16[:, s * 128:(s + 1) * 128], ident[:])
                    mt = mtpool.tile([128, 128], dt.float16)  # [ch, tok]
                    nc.vector.tensor_copy(mt[:], tps[:])
                    ops = po.tile([128, C_OUT], dt.float32)
                    nc.tensor.matmul(ops[:], mt[:], w2T[:],
                                     start=True, stop=True)
                    nc.vector.tensor_add(
                        osb[:, s * C_OUT:(s + 1) * C_OUT], ops[:], b2[:])
                nc.sync.dma_start(
                    out_d.ap()[c * CHUNK:(c + 1) * CHUNK, :]
                    .rearrange("(s p) o -> p s o", p=128),
                    osb[:].rearrange("p (s o) -> p s o", o=C_OUT),
                )

    nc.compile()
    return nc


def _get_program():
    if "nc" not in _CACHE:
        _CACHE["nc"] = _build_program()
    return _CACHE["nc"]


def _host_prep(x, knn_idx, W1, b1, Wc0, bc0, Wc1, bc1, W2, b2):
    """Fuse weights and build per-core input maps."""
    f64 = np.float64
    weff = (Wc0.astype(f64) @ W1.astype(f64))                    # [128, 3]
    beff = (Wc0.astype(f64) @ b1.astype(f64) + bc0.astype(f64))  # [128]
    w2s = W2.astype(f64) / K                                     # fold 1/K

    weffT = np.ascontiguousarray(weff.T.astype(np.float16))
    beff_c = np.ascontiguousarray(beff.astype(np.float32)[:, None])
    wc1T = np.ascontiguousarray(Wc1.T.astype(np.float16))
    bc1_c = np.ascontiguousarray(bc1.astype(np.float32)[:, None])
    w2T = np.ascontiguousarray(w2s.T.astype(np.float16))
    b2_b = np.ascontiguousarray(np.tile(b2.astype(np.float32)[None, :], (128, 1)))
    ident = np.eye(128, dtype=np.float16)
    ones = np.ones((1, 128), dtype=np.float16)

    in_maps = []
    for bi in range(B):
        xT = np.ascontiguousarray(x[bi].T.astype(np.float16))
        # idx layout: per chunk, k-major flat list wrapped into 16
        # partitions, replicated to all 8 Q7 core groups (128 partitions).
        kb = knn_idx[bi].astype(np.int16)
        cols = []
        for c in range(N_CHUNKS):
            flat = np.ascontiguousarray(
                kb[c * CHUNK:(c + 1) * CHUNK, :].T).reshape(-1)  # k-major
            wrapped = flat.reshape(IDX_SLOTS, 16).T
            cols.append(np.tile(wrapped, (8, 1)))
        idx = np.ascontiguousarray(np.concatenate(cols, axis=1))
        in_maps.append({
            "xT": xT, "idx": idx, "weffT": weffT, "beff": beff_c,
            "wc1T": wc1T, "bc1c": bc1_c, "w2T": w2T, "b2b": b2_b,
            "ident": ident, "ones": ones,
        })
    return in_maps


def kernel(x, knn_idx, W1, b1, Wc0, bc0, Wc1, bc1, W2, b2):
    x = np.asarray(x)
    knn_idx = np.asarray(knn_idx)
    args = [np.asarray(a) for a in (W1, b1, Wc0, bc0, Wc1, bc1, W2, b2)]
    in_maps = _host_prep(x, knn_idx, *args)
    nc = _get_program()
    from concourse import bass_utils
    res = bass_utils.run_bass_kernel_spmd(nc, in_maps, core_ids=list(range(B)))
    return np.stack([res.results[i]["out"] for i in range(B)], axis=0)
